# revision 5
# baseline (speedup 1.0000x reference)
"""Trainium2 Bass kernel for byte-to-patch cross attention.

Problem shapes (hardcoded): B=2, S=4096, P=1024, D=1024, H=16 heads, dh=64.

Sharding: 8 cores = batch (2) x head-groups (4). Core i handles batch i//4
and heads 4*(i%4) .. 4*(i%4)+3 (a 256-wide slice of the projection dims).
Each core computes q/k/v projections for its head slice, masked softmax
attention, and its partial output projection  O_g @ wo[:, g].T.  The host
sums the 4 partials per batch (the "all-reduce" of the tensor-parallel
output projection) and adds the bv/bo bias terms.

On-device layout notes:
 - All big operands are staged transposed by the host (contraction dim on
   partitions), so no on-device transposes are needed anywhere.
 - scores are computed transposed: S_h[p, t] = K_h @ Q_h^T, patches on
   partitions, bytes on the free axis.
 - softmax runs over the partition axis: exp on ACT, masking via one
   fp16 scalar_tensor_tensor per tile, denominators via ones-matmuls
   (M=64, col-tiled pairs) that land broadcast across 64 partitions in
   PSUM, then one reciprocal + one multiply normalize during PSUM
   evacuation of the attention output.
 - final projection consumes O^T tiles as the stationary operand and
   writes Y[t, d] naturally.
"""

import sys

sys.path.insert(0, "/opt/trn_rl_repo")

import numpy as np

import concourse.bass as bass
import concourse.mybir as mybir
from concourse import bacc, tile
from concourse.bass_utils import run_bass_kernel_spmd

B, S, P, D, H = 2, 4096, 1024, 1024, 16
HPC = H // 4          # heads per core = 4
GD = HPC * 64         # projection dim slice per core = 256
DH = 64               # head dim
SCALE = 1.0 / 8.0     # 1/sqrt(dh)

F16 = mybir.dt.float16
F32 = mybir.dt.float32

TC = 512              # byte-seq chunk (matmul free dim / psum bank)
NTC = S // TC         # 8
NPT = P // 128        # 8 patch tiles
NK = D // 128         # 8 contraction chunks for projections

_CACHE = {}


def _build_program():
    nc = bacc.Bacc("TRN2", target_bir_lowering=False, debug=False)

    xt_d = nc.dram_tensor("xt", [D, S], F16, kind="ExternalInput")       # X^T
    rt_d = nc.dram_tensor("rt", [D, P], F16, kind="ExternalInput")       # R^T
    wqt_d = nc.dram_tensor("wqt", [D, GD], F16, kind="ExternalInput")    # wq_g^T
    wkt_d = nc.dram_tensor("wkt", [D, GD], F16, kind="ExternalInput")    # wk_g^T
    wvt_d = nc.dram_tensor("wvt", [D, GD], F16, kind="ExternalInput")    # wv_g^T
    wot_d = nc.dram_tensor("wot", [GD, D], F16, kind="ExternalInput")    # wo[:,g]^T
    cbc_d = nc.dram_tensor("cbc", [128, S], F16, kind="ExternalInput")   # cumsum bcast
    jcol_d = nc.dram_tensor("jcol", [128, NPT], F16, kind="ExternalInput")
    bqc_d = nc.dram_tensor("bqc", [128, 2], F32, kind="ExternalInput")
    bkc_d = nc.dram_tensor("bkc", [128, 2], F32, kind="ExternalInput")
    y_d = nc.dram_tensor("y", [S, D], F16, kind="ExternalOutput")

    with tile.TileContext(nc) as tc:
        with (
            tc.tile_pool(name="const", bufs=1) as cpool,
            tc.tile_pool(name="xt", bufs=NK) as xt_pool,
            tc.tile_pool(name="rt", bufs=NK) as rt_pool,
            tc.tile_pool(name="qt", bufs=2) as qt_pool,
            tc.tile_pool(name="kt", bufs=2) as kt_pool,
            tc.tile_pool(name="vp", bufs=NPT) as v_pool,
            tc.tile_pool(name="probs", bufs=20) as pr_pool,
            tc.tile_pool(name="ot", bufs=2) as ot_pool,
            tc.tile_pool(name="misc", bufs=4) as misc_pool,
            tc.tile_pool(name="yout", bufs=4) as y_pool,
            tc.tile_pool(name="ps_proj", bufs=2, space="PSUM") as ps_proj,
            tc.tile_pool(name="ps_sc", bufs=2, space="PSUM") as ps_sc,
            tc.tile_pool(name="ps_pv", bufs=1, space="PSUM") as ps_pv,
        ):
            # ---- constants / weights -------------------------------------
            wqt = [cpool.tile([128, GD], F16, tag=f"wqt{k}", name=f"wqt{k}") for k in range(NK)]
            wkt = [cpool.tile([128, GD], F16, tag=f"wkt{k}", name=f"wkt{k}") for k in range(NK)]
            wvt = [cpool.tile([128, GD], F16, tag=f"wvt{k}", name=f"wvt{k}") for k in range(NK)]
            for k in range(NK):
                nc.sync.dma_start(wqt[k][:], wqt_d[k * 128:(k + 1) * 128, :])
                nc.sync.dma_start(wkt[k][:], wkt_d[k * 128:(k + 1) * 128, :])
                nc.sync.dma_start(wvt[k][:], wvt_d[k * 128:(k + 1) * 128, :])
            wot = [cpool.tile([128, D], F16, tag=f"wot{k}", name=f"wot{k}") for k in range(2)]
            for k in range(2):
                nc.sync.dma_start(wot[k][:], wot_d[k * 128:(k + 1) * 128, :])
            cbc = cpool.tile([128, S], F16, tag="cbc")
            nc.sync.dma_start(cbc[:], cbc_d[:])
            jcol = cpool.tile([128, NPT], F16, tag="jcol")
            nc.sync.dma_start(jcol[:], jcol_d[:])
            bqc = cpool.tile([128, 2], F32, tag="bqc")
            nc.sync.dma_start(bqc[:], bqc_d[:])
            bkc = cpool.tile([128, 2], F32, tag="bkc")
            nc.sync.dma_start(bkc[:], bkc_d[:])
            ones64 = cpool.tile([128, 64], F16, tag="ones64")
            nc.vector.memset(ones64[:], 1.0)

            # ---- K^T, V projections (from R^T) ---------------------------
            rt = [rt_pool.tile([128, P], F16, name="rt_t") for _ in range(NK)]
            for k in range(NK):
                nc.sync.dma_start(rt[k][:], rt_d[k * 128:(k + 1) * 128, :])

            # K^T [GD, P] as 2 sbuf tiles; scale 1/8 and bias folded in.
            kt = [kt_pool.tile([128, P], F16, name="kt_t") for _ in range(2)]
            for m in range(2):
                for pc in range(P // TC):
                    pk = ps_proj.tile([128, TC], F32, tag="pj")
                    for k in range(NK):
                        nc.tensor.matmul(
                            pk[:],
                            wkt[k][:, m * 128:(m + 1) * 128],
                            rt[k][:, pc * TC:(pc + 1) * TC],
                            start=(k == 0),
                            stop=(k == NK - 1),
                        )
                    nc.vector.tensor_scalar(
                        kt[m][:, pc * TC:(pc + 1) * TC], pk[:],
                        bkc[:, m:m + 1], SCALE,
                        op0=mybir.AluOpType.add, op1=mybir.AluOpType.mult,
                    )

            # V [P, GD] natural, 8 tiles of [128, 256]
            vt = [v_pool.tile([128, GD], F16, name="v_t") for _ in range(NPT)]
            for pt in range(NPT):
                pv = ps_proj.tile([128, GD], F32, tag="pj")
                for k in range(NK):
                    nc.tensor.matmul(
                        pv[:],
                        rt[k][:, pt * 128:(pt + 1) * 128],
                        wvt[k][:],
                        start=(k == 0),
                        stop=(k == NK - 1),
                    )
                nc.vector.tensor_copy(vt[pt][:], pv[:])

            # ---- Q^T projection (from X^T) -------------------------------
            xt = [xt_pool.tile([128, S], F16, name="xt_t") for _ in range(NK)]
            for k in range(NK):
                nc.sync.dma_start(xt[k][:], xt_d[k * 128:(k + 1) * 128, :])

            qt = [qt_pool.tile([128, S], F16, name="qt_t") for _ in range(2)]
            for m in range(2):
                for tc_i in range(NTC):
                    pq = ps_proj.tile([128, TC], F32, tag="pj")
                    for k in range(NK):
                        nc.tensor.matmul(
                            pq[:],
                            wqt[k][:, m * 128:(m + 1) * 128],
                            xt[k][:, tc_i * TC:(tc_i + 1) * TC],
                            start=(k == 0),
                            stop=(k == NK - 1),
                        )
                    nc.vector.tensor_scalar_add(
                        qt[m][:, tc_i * TC:(tc_i + 1) * TC], pq[:], bqc[:, m:m + 1]
                    )

            # ---- attention + output projection, per byte-chunk -----------
            ot_tiles = [ot_pool.tile([128, S], F16, name="ot_t") for _ in range(2)]
            for tc_i in range(NTC):
                tsl = slice(tc_i * TC, (tc_i + 1) * TC)
                for g2 in range(2):          # head pair (local heads 2g2, 2g2+1)
                    probs = [[None] * NPT for _ in range(2)]
                    for pt in range(NPT):
                        for hh in range(2):  # head within pair
                            base = hh * 64
                            psc = ps_sc.tile([128, TC], F32, tag="sc")
                            nc.tensor.matmul(
                                psc[:],
                                kt[g2][base:base + 64, pt * 128:(pt + 1) * 128],
                                qt[g2][base:base + 64, tsl],
                                start=True, stop=True,
                                tile_position=(base, 0),
                            )
                            pr = pr_pool.tile([128, TC], F16, tag="pr")
                            # exp then mask: probs = (cbc >= j) * exp(scores)
                            nc.scalar.activation(
                                pr[:], psc[:], mybir.ActivationFunctionType.Exp
                            )
                            nc.vector.scalar_tensor_tensor(
                                pr[:], cbc[:, tsl], jcol[:, pt:pt + 1], pr[:],
                                op0=mybir.AluOpType.is_ge,
                                op1=mybir.AluOpType.mult,
                            )
                            probs[hh][pt] = pr

                    # PV and denominator matmuls (col-tiled head pairs)
                    ppv = ps_pv.tile([128, TC], F32, tag="pv")
                    pden = ps_pv.tile([128, TC], F32, tag="den")
                    for pt in range(NPT):
                        for hh in range(2):
                            lh = 2 * g2 + hh
                            nc.tensor.matmul(
                                ppv[hh * 64:(hh + 1) * 64, :],
                                vt[pt][:, lh * 64:(lh + 1) * 64],
                                probs[hh][pt][:],
                                start=(pt == 0), stop=(pt == NPT - 1),
                                tile_position=(0, hh * 64),
                            )
                            nc.tensor.matmul(
                                pden[hh * 64:(hh + 1) * 64, :],
                                ones64[:],
                                probs[hh][pt][:],
                                start=(pt == 0), stop=(pt == NPT - 1),
                                tile_position=(0, hh * 64),
                            )
                    recip = misc_pool.tile([128, TC], F32, tag="recip")
                    nc.vector.reciprocal(recip[:], pden[:])
                    nc.vector.tensor_mul(
                        ot_tiles[g2][:, tsl], ppv[:], recip[:]
                    )

                # output projection for this byte chunk (4 t-tiles of 128)
                for tt in range(4):
                    t0 = tc_i * TC + tt * 128
                    for n in range(2):
                        py = ps_proj.tile([128, TC], F32, tag="pj")
                        for k2 in range(2):
                            nc.tensor.matmul(
                                py[:],
                                ot_tiles[k2][:, t0:t0 + 128],
                                wot[k2][:, n * TC:(n + 1) * TC],
                                start=(k2 == 0), stop=(k2 == 1),
                            )
                        ysb = y_pool.tile([128, TC], F16, tag="y")
                        nc.vector.tensor_copy(ysb[:], py[:])
                        nc.sync.dma_start(
                            y_d[t0:t0 + 128, n * TC:(n + 1) * TC], ysb[:]
                        )

    nc.compile()
    return nc


def _get_program():
    if "nc" not in _CACHE:
        _CACHE["nc"] = _build_program()
    return _CACHE["nc"]


def _prep_inputs(queries, patch_representations, patch_boundaries,
                 wq, wk, wv, wo, bq, bk):
    """Build the 8 per-core input maps."""
    in_maps = []
    jcol = (np.arange(128, dtype=np.float32)[:, None]
            + 128.0 * np.arange(NPT, dtype=np.float32)[None, :]).astype(np.float16)
    for core in range(8):
        b, g = core // 4, core % 4
        sl = slice(g * GD, (g + 1) * GD)
        c = np.cumsum(patch_boundaries[b]).astype(np.float32)
        cbc = np.broadcast_to(c.astype(np.float16), (128, S)).copy()
        in_maps.append({
            "xt": np.ascontiguousarray(queries[b].T).astype(np.float16),
            "rt": np.ascontiguousarray(patch_representations[b].T).astype(np.float16),
            "wqt": np.ascontiguousarray(wq[sl, :].T).astype(np.float16),
            "wkt": np.ascontiguousarray(wk[sl, :].T).astype(np.float16),
            "wvt": np.ascontiguousarray(wv[sl, :].T).astype(np.float16),
            "wot": np.ascontiguousarray(wo[:, sl].T).astype(np.float16),
            "cbc": cbc,
            "jcol": jcol,
            "bqc": np.ascontiguousarray(bq[sl].reshape(2, 128).T).astype(np.float32),
            "bkc": np.ascontiguousarray(bk[sl].reshape(2, 128).T).astype(np.float32),
        })
    return in_maps


def _reduce_outputs(results, wo, bv, bo):
    y = np.zeros((B, S, D), dtype=np.float32)
    for core in range(8):
        y[core // 4] += results[core]["y"].astype(np.float32)
    y += (bv @ wo.T + bo)[None, None, :]
    return y


def kernel(queries, patch_representations, patch_boundaries,
           wq, wk, wv, wo, bq, bk, bv, bo):
    nc = _get_program()
    in_maps = _prep_inputs(queries, patch_representations, patch_boundaries,
                           wq, wk, wv, wo, bq, bk)
    res = run_bass_kernel_spmd(nc, in_maps, core_ids=list(range(8)))
    return _reduce_outputs(res.results, wo, bv, bo)


# revision 11
# speedup vs baseline: 12067.2049x; 12067.2049x over previous
"""Trainium2 Bass kernel for byte-to-patch cross attention.

Problem shapes (hardcoded): B=2, S=4096, P=1024, D=1024, H=16 heads, dh=64.

Sharding: 8 cores = batch (2) x head-groups (4). Core i handles batch i//4
and heads 4*(i%4) .. 4*(i%4)+3 (a 256-wide slice of the projection dims).
Each core computes q/k/v projections for its head slice, masked softmax
attention, and its partial output projection  O_g @ wo[:, g].T.  The host
sums the 4 partials per batch (the "all-reduce" of the tensor-parallel
output projection) and adds the bv/bo bias terms.

On-device layout notes:
 - All big operands are staged transposed by the host (contraction dim on
   partitions), so no on-device transposes are needed anywhere.
 - scores are computed transposed: S_h[p, t] = K_h @ Q_h^T, patches on
   partitions, bytes on the free axis.
 - softmax runs over the partition axis: exp on ACT, masking via one
   fp16 scalar_tensor_tensor per tile, denominators via ones-matmuls
   (M=64, col-tiled pairs) that land broadcast across 64 partitions in
   PSUM, then one reciprocal + one multiply normalize during PSUM
   evacuation of the attention output.
 - final projection consumes O^T tiles as the stationary operand and
   writes Y[t, d] naturally.
"""

import sys

sys.path.insert(0, "/opt/trn_rl_repo")

import numpy as np

import concourse.bass as bass
import concourse.mybir as mybir
from concourse import bacc, tile
from concourse.bass_utils import run_bass_kernel_spmd

B, S, P, D, H = 2, 4096, 1024, 1024, 16
HPC = H // 4          # heads per core = 4
GD = HPC * 64         # projection dim slice per core = 256
DH = 64               # head dim
SCALE = 1.0 / 8.0     # 1/sqrt(dh)

F16 = mybir.dt.float16
F32 = mybir.dt.float32

TC = 512              # byte-seq chunk (matmul free dim / psum bank)
NTC = S // TC         # 8
NPT = P // 128        # 8 patch tiles
NK = D // 128         # 8 contraction chunks for projections

_CACHE = {}


def _build_program(vis=None, repeat=1):
    nc = bacc.Bacc("TRN2", target_bir_lowering=False, debug=False)

    xt_d = nc.dram_tensor("xt", [D, S], F16, kind="ExternalInput")       # X^T
    rt_d = nc.dram_tensor("rt", [D, P], F16, kind="ExternalInput")       # R^T
    wqt_d = nc.dram_tensor("wqt", [D, GD], F16, kind="ExternalInput")    # wq_g^T
    wkt_d = nc.dram_tensor("wkt", [D, GD], F16, kind="ExternalInput")    # wk_g^T
    wvt_d = nc.dram_tensor("wvt", [D, GD], F16, kind="ExternalInput")    # wv_g^T
    wot_d = nc.dram_tensor("wot", [GD, D], F16, kind="ExternalInput")    # wo[:,g]^T
    cbc_d = nc.dram_tensor("cbc", [128, S], F16, kind="ExternalInput")   # cumsum bcast
    jcol_d = nc.dram_tensor("jcol", [128, NPT], F16, kind="ExternalInput")
    bqc_d = nc.dram_tensor("bqc", [128, 2], F32, kind="ExternalInput")
    bkc_d = nc.dram_tensor("bkc", [128, 2], F32, kind="ExternalInput")
    y_d = nc.dram_tensor("y", [S, D], F16, kind="ExternalOutput")

    with tile.TileContext(nc) as tc:
        with (
            tc.tile_pool(name="const", bufs=1) as cpool,
            tc.tile_pool(name="xt", bufs=1) as xt_pool,
            tc.tile_pool(name="rt", bufs=1) as rt_pool,
            tc.tile_pool(name="qt", bufs=2) as qt_pool,
            tc.tile_pool(name="kt", bufs=2) as kt_pool,
            tc.tile_pool(name="vp", bufs=NPT) as v_pool,
            tc.tile_pool(name="probs", bufs=20) as pr_pool,
            tc.tile_pool(name="ot", bufs=2) as ot_pool,
            tc.tile_pool(name="misc", bufs=4) as misc_pool,
            tc.tile_pool(name="yout", bufs=4) as y_pool,
            tc.tile_pool(name="ps_proj", bufs=2, space="PSUM") as ps_proj,
            tc.tile_pool(name="ps_sc", bufs=2, space="PSUM") as ps_sc,
            tc.tile_pool(name="ps_pv", bufs=2, space="PSUM") as ps_pv,
        ):
          if vis is None:
            vis = [[1] * NPT for _ in range(NTC)]
          for _rep in range(repeat):
            # ---- merged strided loads (1 DMA per tensor) ------------------
            # dram [(k p), c] -> sbuf [p, (k c)]
            rt_sb = rt_pool.tile([128, NK * P], F16, name="rt_sb")
            nc.sync.dma_start(
                rt_sb[:].rearrange("p (k c) -> p k c", k=NK),
                rt_d.rearrange("(k p) c -> p k c", p=128))
            wk_sb = cpool.tile([128, NK * GD], F16, tag="wk_sb")
            nc.sync.dma_start(
                wk_sb[:].rearrange("p (k c) -> p k c", k=NK),
                wkt_d.rearrange("(k p) c -> p k c", p=128))
            wv_sb = cpool.tile([128, NK * GD], F16, tag="wv_sb")
            nc.sync.dma_start(
                wv_sb[:].rearrange("p (k c) -> p k c", k=NK),
                wvt_d.rearrange("(k p) c -> p k c", p=128))
            wq_sb = cpool.tile([128, NK * GD], F16, tag="wq_sb")
            nc.sync.dma_start(
                wq_sb[:].rearrange("p (k c) -> p k c", k=NK),
                wqt_d.rearrange("(k p) c -> p k c", p=128))
            wqt = [wq_sb[:, k * GD:(k + 1) * GD] for k in range(NK)]
            wkt = [wk_sb[:, k * GD:(k + 1) * GD] for k in range(NK)]
            wvt = [wv_sb[:, k * GD:(k + 1) * GD] for k in range(NK)]
            rt = [rt_sb[:, k * P:(k + 1) * P] for k in range(NK)]
            wo_sb = cpool.tile([128, 2 * D], F16, tag="wo_sb")
            nc.sync.dma_start(
                wo_sb[:].rearrange("p (k c) -> p k c", k=2),
                wot_d.rearrange("(k p) c -> p k c", p=128))
            wot = [wo_sb[:, k * D:(k + 1) * D] for k in range(2)]
            cbc = cpool.tile([128, S], F16, tag="cbc")
            nc.sync.dma_start(cbc[:], cbc_d[:])
            jcol = cpool.tile([128, NPT], F16, tag="jcol")
            nc.sync.dma_start(jcol[:], jcol_d[:])
            bqc = cpool.tile([128, 2], F32, tag="bqc")
            nc.sync.dma_start(bqc[:], bqc_d[:])
            bkc = cpool.tile([128, 2], F32, tag="bkc")
            nc.sync.dma_start(bkc[:], bkc_d[:])
            ones64 = cpool.tile([128, 64], F16, tag="ones64")
            nc.vector.memset(ones64[:], 1.0)

            # ---- K^T, V projections (from R^T) ---------------------------
            # K^T [GD, P] as 2 sbuf tiles; scale 1/8 and bias folded in.
            kt = [kt_pool.tile([128, P], F16, name="kt_t") for _ in range(2)]
            for m in range(2):
                for pc in range(P // TC):
                    pk = ps_proj.tile([128, TC], F32, tag="pj")
                    for k in range(NK):
                        nc.tensor.matmul(
                            pk[:],
                            wkt[k][:, m * 128:(m + 1) * 128],
                            rt[k][:, pc * TC:(pc + 1) * TC],
                            start=(k == 0),
                            stop=(k == NK - 1),
                        )
                    nc.vector.tensor_scalar(
                        kt[m][:, pc * TC:(pc + 1) * TC], pk[:],
                        bkc[:, m:m + 1], SCALE,
                        op0=mybir.AluOpType.add, op1=mybir.AluOpType.mult,
                    )

            # V [P, GD] natural, 8 tiles of [128, 256]
            vt = [v_pool.tile([128, GD], F16, name="v_t") for _ in range(NPT)]
            for pt in range(NPT):
                pv = ps_proj.tile([128, GD], F32, tag="pj")
                for k in range(NK):
                    nc.tensor.matmul(
                        pv[:],
                        rt[k][:, pt * 128:(pt + 1) * 128],
                        wvt[k],
                        start=(k == 0),
                        stop=(k == NK - 1),
                    )
                nc.vector.tensor_copy(vt[pt][:], pv[:])

            # ---- Q^T projection (from X^T) -------------------------------
            xt_sb = xt_pool.tile([128, NK * S], F16, name="xt_sb")
            nc.sync.dma_start(
                xt_sb[:].rearrange("p (k c) -> p k c", k=NK),
                xt_d.rearrange("(k p) c -> p k c", p=128))
            xt = [xt_sb[:, k * S:(k + 1) * S] for k in range(NK)]

            qt = [qt_pool.tile([128, S], F16, name="qt_t") for _ in range(2)]
            for m in range(2):
                for tc_i in range(NTC):
                    pq = ps_proj.tile([128, TC], F32, tag="pj")
                    for k in range(NK):
                        nc.tensor.matmul(
                            pq[:],
                            wqt[k][:, m * 128:(m + 1) * 128],
                            xt[k][:, tc_i * TC:(tc_i + 1) * TC],
                            start=(k == 0),
                            stop=(k == NK - 1),
                        )
                    nc.vector.tensor_scalar_add(
                        qt[m][:, tc_i * TC:(tc_i + 1) * TC], pq[:], bqc[:, m:m + 1]
                    )

            # ---- attention + output projection, per byte-chunk -----------
            ot_tiles = [ot_pool.tile([128, S], F16, name="ot_t") for _ in range(2)]
            for tc_i in range(NTC):
                tsl = slice(tc_i * TC, (tc_i + 1) * TC)
                for g2 in range(2):          # head pair (local heads 2g2, 2g2+1)
                    probs = [[None] * NPT for _ in range(2)]
                    live_pt = [pt for pt in range(NPT) if vis[tc_i][pt] > 0]
                    for pt in live_pt:
                        for hh in range(2):  # head within pair
                            base = hh * 64
                            psc = ps_sc.tile([128, TC], F32, tag="sc")
                            nc.tensor.matmul(
                                psc[:],
                                kt[g2][base:base + 64, pt * 128:(pt + 1) * 128],
                                qt[g2][base:base + 64, tsl],
                                start=True, stop=True,
                                tile_position=(base, 0),
                            )
                            pr = pr_pool.tile([128, TC], F16, tag="pr")
                            # exp then mask: probs = (cbc >= j) * exp(scores)
                            nc.scalar.activation(
                                pr[:], psc[:], mybir.ActivationFunctionType.Exp
                            )
                            if vis[tc_i][pt] == 1:
                                nc.vector.scalar_tensor_tensor(
                                    pr[:], cbc[:, tsl], jcol[:, pt:pt + 1], pr[:],
                                    op0=mybir.AluOpType.is_ge,
                                    op1=mybir.AluOpType.mult,
                                )
                            probs[hh][pt] = pr

                    # PV and denominator matmuls (col-tiled head pairs)
                    ppv = ps_pv.tile([128, TC], F32, tag="pv")
                    pden = ps_pv.tile([128, TC], F32, tag="den")
                    for pt in live_pt:
                        for hh in range(2):
                            lh = 2 * g2 + hh
                            nc.tensor.matmul(
                                ppv[hh * 64:(hh + 1) * 64, :],
                                vt[pt][:, lh * 64:(lh + 1) * 64],
                                probs[hh][pt][:],
                                start=(pt == live_pt[0]), stop=(pt == live_pt[-1]),
                                tile_position=(0, hh * 64),
                            )
                            nc.tensor.matmul(
                                pden[hh * 64:(hh + 1) * 64, :],
                                ones64[:],
                                probs[hh][pt][:],
                                start=(pt == live_pt[0]), stop=(pt == live_pt[-1]),
                                tile_position=(0, hh * 64),
                            )
                    recip = misc_pool.tile([128, TC], F32, tag="recip")
                    nc.vector.reciprocal(recip[:], pden[:])
                    nc.vector.tensor_mul(
                        ot_tiles[g2][:, tsl], ppv[:], recip[:]
                    )

                # output projection for this byte chunk (4 t-tiles of 128)
                for tt in range(4):
                    t0 = tc_i * TC + tt * 128
                    ysb = y_pool.tile([128, D], F16, tag="y")
                    for n in range(2):
                        py = ps_proj.tile([128, TC], F32, tag="pj")
                        for k2 in range(2):
                            nc.tensor.matmul(
                                py[:],
                                ot_tiles[k2][:, t0:t0 + 128],
                                wot[k2][:, n * TC:(n + 1) * TC],
                                start=(k2 == 0), stop=(k2 == 1),
                            )
                        nc.vector.tensor_copy(ysb[:, n * TC:(n + 1) * TC], py[:])
                    nc.sync.dma_start(y_d[t0:t0 + 128, :], ysb[:])

    nc.compile()
    return nc


def _vis_plan(patch_boundaries):
    """vis[tc][pt]: 0 = fully masked in every batch (skip), 2 = fully
    visible in every batch (no mask op), 1 = boundary (apply mask).
    Must be valid for all cores, i.e. union over batches."""
    cs = np.cumsum(patch_boundaries, axis=1)  # [B, S]
    vis = []
    for tci in range(NTC):
        lo = cs[:, tci * TC].min()
        hi = cs[:, (tci + 1) * TC - 1].max()
        row = []
        for pt in range(NPT):
            if pt * 128 > hi:
                row.append(0)
            elif (pt + 1) * 128 - 1 <= lo:
                row.append(2)
            else:
                row.append(1)
        vis.append(row)
    return vis


def _get_program(vis=None, repeat=1):
    key = (tuple(tuple(r) for r in vis) if vis else None, repeat)
    if key not in _CACHE:
        _CACHE[key] = _build_program(vis, repeat)
    return _CACHE[key]


def _prep_inputs(queries, patch_representations, patch_boundaries,
                 wq, wk, wv, wo, bq, bk):
    """Build the 8 per-core input maps."""
    in_maps = []
    jcol = (np.arange(128, dtype=np.float32)[:, None]
            + 128.0 * np.arange(NPT, dtype=np.float32)[None, :]).astype(np.float16)
    for core in range(8):
        b, g = core // 4, core % 4
        sl = slice(g * GD, (g + 1) * GD)
        c = np.cumsum(patch_boundaries[b]).astype(np.float32)
        cbc = np.broadcast_to(c.astype(np.float16), (128, S)).copy()
        in_maps.append({
            "xt": np.ascontiguousarray(queries[b].T).astype(np.float16),
            "rt": np.ascontiguousarray(patch_representations[b].T).astype(np.float16),
            "wqt": np.ascontiguousarray(wq[sl, :].T).astype(np.float16),
            "wkt": np.ascontiguousarray(wk[sl, :].T).astype(np.float16),
            "wvt": np.ascontiguousarray(wv[sl, :].T).astype(np.float16),
            "wot": np.ascontiguousarray(wo[:, sl].T).astype(np.float16),
            "cbc": cbc,
            "jcol": jcol,
            "bqc": np.ascontiguousarray(bq[sl].reshape(2, 128).T).astype(np.float32),
            "bkc": np.ascontiguousarray(bk[sl].reshape(2, 128).T).astype(np.float32),
        })
    return in_maps


def _reduce_outputs(results, wo, bv, bo):
    y = np.zeros((B, S, D), dtype=np.float32)
    for core in range(8):
        y[core // 4] += results[core]["y"].astype(np.float32)
    y += (bv @ wo.T + bo)[None, None, :]
    return y


def kernel(queries, patch_representations, patch_boundaries,
           wq, wk, wv, wo, bq, bk, bv, bo):
    queries = np.asarray(queries, dtype=np.float32)
    patch_representations = np.asarray(patch_representations, dtype=np.float32)
    patch_boundaries = np.asarray(patch_boundaries)
    wq, wk, wv, wo = (np.asarray(a, dtype=np.float32) for a in (wq, wk, wv, wo))
    bq, bk, bv, bo = (np.asarray(a, dtype=np.float32) for a in (bq, bk, bv, bo))
    nc = _get_program(_vis_plan(patch_boundaries))
    in_maps = _prep_inputs(queries, patch_representations, patch_boundaries,
                           wq, wk, wv, wo, bq, bk)
    res = run_bass_kernel_spmd(nc, in_maps, core_ids=list(range(8)))
    return _reduce_outputs(res.results, wo, bv, bo)
